# revision 1
# baseline (speedup 1.0000x reference)
"""Trainium2 Bass kernel for nn_Attention (B=4, N=2048, D=1024, H=8 heads).

Computes: qkv = x @ Wkv.T; q,k,v split into 8 heads of 128 dims;
y = softmax(q k^T / sqrt(128) + mask) v;  out = y @ Wo.T + bo.

Sharding (8 NeuronCores): core (b, g) = batch b in 0..3, head-group g in 0..1
(4 heads each).  Each core computes its 4 heads' attention and a partial
output projection; the host sums the two head-group partials per batch and
adds bo.

The additive mask is skipped on device: the problem spec fills it with zeros
(exp(s + 0) == exp(s)).  If a nonzero mask is ever passed, kernel() falls back
to an exact numpy implementation.

Device-side design (v2):
 - All matmul operands are 16-bit (full PE rate): x/Wkv in bf16, q/k/v/
   exp-tiles/Wo in fp16.  Accumulation stays fp32 in PSUM.  ~4e-3 final
   rel err (gate is 2e-2).
 - The host pre-transposes AND pre-casts x, Wkv and Wo slices per core,
   so the device does NO weight/x transposes and no casts: 16-bit inputs
   DMA straight into the persistent SBUF tiles (half the DMA bytes).
   The output is written fp16 (host sums the two head-group partials in
   fp32).
 - Scores are computed TRANSPOSED (sT[k, q] = kT_tile.T @ qT) so softmax
   needs no p transposes before the PV matmul.
 - exp runs on the scalar engine reading scores from PSUM with the
   1/sqrt(128) scale fused in, writing fp16 tiles to SBUF.
 - The softmax denominator is a running fp16 elementwise sum of the 16
   exp tiles on DVE (fast 2-byte mode), finished by a single ones-matmul
   for the cross-partition reduction -- this removes the 1/3 of attention
   matmul work the old kernel spent on per-tile ones-matmuls.
 - PE emission is software-pipelined: the next head's QKV projection
   matmuls (and the output projection at the tail) are interleaved into
   the attention k-loop as fillers, so the PE never drains while the
   scalar engine works through the exp stream.  Head 0's projection is
   emitted in x-DMA arrival order; head 3's q-half1 projection fills its
   own qc0 block.
 - PSUM budget (8 banks): scores 2x[128,1024]f32 (4) + yacc (2) +
   projection/oproj/transpose scratch (2).  qT/kT/vT/v_nat double-buffer
   on head parity so interleaved projection never races attention.
"""

import numpy as np

B, N, D, H = 4, 2048, 1024, 8
HD = D // H          # 128 head dim
HPC = H // 2         # 4 heads per core
DY = HPC * HD        # 512 local y dims per core
P = 128
NT = N // P          # 16 n-tiles
DC = D // P          # 8 d-chunks
KT = N // P          # 16 k-tiles
QC = 2               # q chunks per head
QW = N // QC         # 1024 q width
MM = 512             # max fp32 moving free dim
NS = 4               # x load n-slices
SW = N // NS         # 512 slice width
SCALE = float(1.0 / np.sqrt(HD))

_CACHE = {}


def _build():
    from contextlib import ExitStack

    import concourse.bacc as bacc
    import concourse.bass as bass
    import concourse.mybir as mybir
    from concourse.masks import make_identity
    from concourse.tile import TileContext

    ts = bass.ts
    F32 = mybir.dt.float32
    F16 = mybir.dt.float16
    BF16 = mybir.dt.bfloat16
    EXP = mybir.ActivationFunctionType.Exp

    nc = bacc.Bacc("TRN2", target_bir_lowering=False, debug=False)
    # Host-pre-transposed, host-pre-cast inputs (see make_in_maps):
    #   xt[d, n]          = x[b][n, d]                   (bf16)
    #   wkvt[d, h*384+c*128+i] = Wkv[c*D + g*DY + h*HD + i, d]  (bf16)
    #   wot[p, h*D + e]   = Wo[e, g*DY + h*HD + p]       (fp16, [DY, D] h-major)
    # 16-bit inputs halve the input DMA bytes and land directly in the
    # persistent SBUF tiles -- no staging or on-device casts needed.
    xt = nc.dram_tensor("xt", [D, N], BF16, kind="ExternalInput")
    wkvt = nc.dram_tensor("wkvt", [D, 3 * DY], BF16, kind="ExternalInput")
    wot = nc.dram_tensor("wot", [DY, D], F16, kind="ExternalInput")
    # fp16 output halves the output DMA; the host sums the two head-group
    # partials in fp32 (fp16 rounding of a partial adds ~5e-4 rel err).
    out = nc.dram_tensor("out", [N, D], F16, kind="ExternalOutput")

    with TileContext(nc) as tc, ExitStack() as top:
        consts = top.enter_context(tc.tile_pool(name="consts", bufs=1))
        ident = consts.tile([P, P], F16, tag="ident")
        make_identity(nc, ident)
        ones32 = consts.tile([P, P], F32, tag="ones32")
        nc.vector.memset(ones32, 1.0)
        ones16 = consts.tile([P, P], F16, tag="ones16")
        nc.vector.tensor_copy(out=ones16, in_=ones32)

        persist = top.enter_context(tc.tile_pool(name="persist", bufs=1))
        xTf = persist.tile([P, DC, N], BF16, tag="xTf")
        wkvTf = persist.tile([P, DC, 3 * DY], BF16, tag="wkvTf")
        woTf = persist.tile([P, HPC, D], F16, tag="woTf")
        # qT/kT/vT/v_nat are double-buffered on head parity so head h+1's
        # projection (interleaved into head h's attention) never overwrites
        # tiles attention is still reading.
        qT = [persist.tile([P, N], F16, tag=f"qT{i}", name=f"qT{i}") for i in range(2)]
        kT = [persist.tile([P, N], F16, tag=f"kT{i}", name=f"kT{i}") for i in range(2)]
        vT = [persist.tile([P, N], F16, tag=f"vT{i}", name=f"vT{i}") for i in range(2)]
        v_nat = [
            persist.tile([P, N], F16, tag=f"vn{i}", name=f"vn{i}") for i in range(2)
        ]
        yT = persist.tile([P, HPC, N], F16, tag="yT")

        work = top.enter_context(tc.tile_pool(name="work", bufs=1))
        psum = top.enter_context(tc.tile_pool(name="ps", bufs=1, space="PSUM"))

        # ---------------- loaders (direct DMA into SBUF tiles) ----------------
        def load_x_slice(ns):
            for j in range(DC):
                nc.sync.dma_start(
                    out=xTf[:, j, ts(ns, SW)], in_=xt.ap()[ts(j, P), ts(ns, SW)]
                )

        def load_wkv(h):
            for j in range(DC):
                nc.sync.dma_start(
                    out=wkvTf[:, j, ts(h, 3 * HD)],
                    in_=wkvt.ap()[ts(j, P), ts(h, 3 * HD)],
                )

        def load_wo():
            for h in range(HPC):
                nc.sync.dma_start(out=woTf[:, h, :], in_=wot.ap()[ts(h, P), :])

        # ------------- projection thunks for head h -------------
        # Each thunk emits one PE instruction (plus trailing DVE drains),
        # so attention blocks can interleave them as fillers.  Tag "st" psum
        # tiles are double-buffered (bufs=2), tag "pp" single (bufs=1).
        def _bufs(tag):
            return 2 if tag == "st" else 1

        def proj_thunks(h, pp_tags, arrival_order=False, split_late=False):
            """Build one-PE-instruction thunks for head h's QKV projection.

            arrival_order: emit (half, nch, c, j) so matmuls match the DMA
            arrival order of x n-slices (head 0's dedicated stretch).
            split_late: return (main, late) where `late` holds the q half-1
            matmuls, which only head h's qc1 attention needs -- they can
            fill head h's own qc0 block.
            """
            hb = h % 2
            cells = {}
            tag_of = {}
            if arrival_order:
                # j-major emission keeps all three of a half's psum tiles
                # live at once: c0 -> "pp", c1/c2 -> the two "st" buffers.
                for c in range(3):
                    for half in range(2):
                        tag_of[(c, half)] = "pp" if c == 0 else "st"
            else:
                for idx, key in enumerate(
                    (c, half) for half in range(2) for c in range(3)
                ):
                    tag_of[key] = pp_tags[idx % len(pp_tags)]
            dests = {0: qT[hb], 1: kT[hb], 2: vT[hb]}

            def mk(c, half, nch, jj):
                def emit():
                    key = (c, half)
                    if key not in cells:
                        cells[key] = psum.tile(
                            [P, QW], F32, tag=tag_of[key], bufs=_bufs(tag_of[key]),
                            name=f"pp{h}{c}{half}",
                        )
                    pp = cells[key]
                    nc.tensor.matmul(
                        pp[:, ts(nch, MM)],
                        wkvTf[:, jj, h * 3 * HD + c * P : h * 3 * HD + (c + 1) * P],
                        xTf[:, jj, half * QW + nch * MM : half * QW + (nch + 1) * MM],
                        start=(jj == 0),
                        stop=(jj == DC - 1),
                    )
                    if jj == DC - 1 and nch == 1:
                        nc.vector.tensor_copy(
                            out=dests[c][:, ts(half, QW)], in_=pp
                        )
                return emit

            def group(c, half):
                return [mk(c, half, nch, jj) for nch in range(2) for jj in range(DC)]

            def tpose(kq):
                cell = {}

                def mkt(k4):
                    def emit():
                        if "pt" not in cell:
                            cell["pt"] = psum.tile(
                                [P, 4, P], F16, tag="pp", bufs=1, name=f"pt{h}{kq}"
                            )
                        pt = cell["pt"]
                        k = kq * 4 + k4
                        nc.tensor.transpose(pt[:, k4], vT[hb][:, ts(k, P)], ident)
                        if k4 == 3:
                            nc.vector.tensor_copy(
                                out=v_nat[hb][:, kq * 4 * P : (kq + 1) * 4 * P],
                                in_=pt.rearrange("p a b -> p (a b)"),
                            )
                    return emit

                return [mkt(k4) for k4 in range(4)]

            transposes = [t for kq in range(4) for t in tpose(kq)]
            if arrival_order:
                thunks = [
                    mk(c, half, nch, jj)
                    for half in range(2)
                    for nch in range(2)
                    for jj in range(DC)
                    for c in range(3)
                ]
                return thunks + transposes
            if split_late:
                main = (
                    group(0, 0) + group(1, 0) + group(2, 0)
                    + group(1, 1) + group(2, 1) + transposes
                )
                return main, group(0, 1)
            return (
                group(0, 0) + group(1, 0) + group(2, 0)
                + group(0, 1) + group(1, 1) + group(2, 1) + transposes
            )

        # ------------- output projection thunks (n-tile i) -------------
        def oproj_thunks(i, tag):
            thunks = []
            cell = {}

            def mk(eh, hh, cell=cell, i=i, tag=tag):
                def emit():
                    if "po" not in cell:
                        cell["po"] = psum.tile(
                            [P, D], F32, tag=tag, bufs=_bufs(tag),
                            name=f"po{i}",
                        )
                    po = cell["po"]
                    nc.tensor.matmul(
                        po[:, ts(eh, MM)],
                        yT[:, hh, ts(i, P)],
                        woTf[:, hh, eh * MM : (eh + 1) * MM],
                        start=(hh == 0),
                        stop=(hh == HPC - 1),
                    )
                    if eh == 1 and hh == HPC - 1:
                        ot = work.tile([P, D], F16, tag="so", bufs=3, name=f"ot{i}")
                        nc.scalar.copy(out=ot, in_=po)
                        nc.sync.dma_start(out=out.ap()[ts(i, P), :], in_=ot)
                return emit

            for eh in range(2):
                for hh in range(HPC):
                    thunks.append(mk(eh, hh))
            return thunks

        # ------------- attention block for (head h, q-chunk qc) -------------
        def attention(h, qc, fillers, nfill, deferred=None):
            """One attention block.  Returns a 'finisher' closure (den
            cross-partition reduce + normalize) that the CALLER emits inside
            the NEXT block (at k==1) -- emitting it here would head-of-line
            block the next block's scores behind the final DVE den-add."""
            hb = h % 2
            yacc = psum.tile([P, QW], F32, tag="acc", bufs=1, name=f"yacc{h}{qc}")
            dacc = None
            ets = []
            for k in range(KT):
                st = psum.tile([P, QW], F32, tag="st", bufs=2, name=f"st{h}{qc}{k}")
                for m in range(2):
                    nc.tensor.matmul(
                        st[:, ts(m, MM)],
                        kT[hb][:, ts(k, P)],
                        qT[hb][:, qc * QW + m * MM : qc * QW + (m + 1) * MM],
                        start=True,
                        stop=True,
                    )
                et = work.tile([P, QW], F16, tag="et", bufs=4, name=f"et{h}{qc}{k}")
                nc.scalar.activation(out=et, in_=st, func=EXP, scale=SCALE)
                if k == 1 and deferred is not None:
                    deferred()
                # fillers run while the scalar engine works through exp
                for _ in range(nfill):
                    if fillers:
                        fillers.popleft()()
                for m in range(2):
                    nc.tensor.matmul(
                        yacc[:, ts(m, MM)],
                        v_nat[hb][:, ts(k, P)],
                        et[:, ts(m, MM)],
                        start=(k == 0),
                        stop=(k == KT - 1),
                    )
                # denominator: running fp16 sum of exp tiles on DVE
                if k == 0:
                    ets.append(et)
                elif k == 1:
                    dacc = work.tile([P, QW], F16, tag="dacc", bufs=2, name=f"da{h}{qc}{k}")
                    nc.vector.tensor_add(out=dacc, in0=ets[0], in1=et)
                else:
                    nd = work.tile([P, QW], F16, tag="dacc", bufs=2, name=f"da{h}{qc}{k}")
                    nc.vector.tensor_add(out=nd, in0=dacc, in1=et)
                    dacc = nd
            # Drain yacc now (scalar engine) so the next block's PV can
            # reuse the accumulation bank promptly.
            ysb = work.tile([P, QW], F32, tag="ysb", bufs=2, name=f"ysb{h}{qc}")
            nc.scalar.copy(out=ysb, in_=yacc)
            dacc_f = dacc

            def finisher():
                dmm = psum.tile([P, QW], F32, tag="st", bufs=2, name=f"dmm{h}{qc}")
                for m in range(2):
                    nc.tensor.matmul(
                        dmm[:, ts(m, MM)], ones16, dacc_f[:, ts(m, MM)],
                        start=True, stop=True,
                    )
                rc = work.tile([P, QW], F32, tag="rc", bufs=2, name=f"rc{h}{qc}")
                nc.vector.reciprocal_approx_fast(out=rc, in_=dmm)
                nc.vector.tensor_mul(out=yT[:, h, ts(qc, QW)], in0=ysb, in1=rc)

            return finisher

        # ---------------- emission schedule ----------------
        from collections import deque

        # interleave the first wkv/x chunk DMAs so the PE's first matmul
        # (which needs wkv j and x j of slice 0) starts as early as possible
        for j in range(DC):
            nc.sync.dma_start(
                out=wkvTf[:, j, 0 : 3 * HD], in_=wkvt.ap()[ts(j, P), 0 : 3 * HD]
            )
            nc.sync.dma_start(out=xTf[:, j, 0:SW], in_=xt.ap()[ts(j, P), 0:SW])
        load_x_slice(1)
        # head 0 projection is a dedicated stretch (overlaps input DMA);
        # double-buffer its psum tiles across the two free tags, and emit
        # matmuls in x-DMA arrival order (n-slice-major) so the PE starts
        # as soon as data lands.
        h0 = deque(proj_thunks(0, pp_tags=("pp", "st"), arrival_order=True))
        emitted = 0
        while h0:
            if emitted == 8:
                load_x_slice(2)
            elif emitted == 24:
                load_x_slice(3)
            elif emitted == 60:
                load_wkv(1)
            h0.popleft()()
            emitted += 1

        late3 = deque()
        fin = None
        for h in range(HPC):
            if h + 1 < HPC:
                if h + 2 == HPC:
                    main, late = proj_thunks(h + 1, pp_tags=("pp",), split_late=True)
                    fillers = deque(main)
                    late3 = deque(late)
                else:
                    fillers = deque(proj_thunks(h + 1, pp_tags=("pp",)))
                if h + 2 < HPC:
                    load_wkv(h + 2)
                if h == 1:
                    load_wo()
                fin = attention(h, 0, fillers, nfill=2, deferred=fin)
                fin = attention(h, 1, fillers, nfill=2, deferred=fin)
                while fillers:
                    fillers.popleft()()
            else:
                # last head: its own q-half1 projection fills qc0; the first
                # half of the output projection fills qc1; the rest follows.
                fin = attention(h, 0, late3, nfill=1, deferred=fin)
                op = deque()
                for i in range(NT // 2):
                    op.extend(oproj_thunks(i, tag="pp"))
                fin = attention(h, 1, op, nfill=3, deferred=fin)
                fin()
                while op:
                    op.popleft()()
                for i in range(NT // 2, NT):
                    for t in oproj_thunks(i, tag="st"):
                        t()
    nc.finalize()
    return nc


def _get_nc():
    if "nc" not in _CACHE:
        _CACHE["nc"] = _build()
    return _CACHE["nc"]


def make_in_maps(x, Wkv, Wo):
    """Per-core input dicts for core = 2*b + g (host pre-transposes + casts)."""
    from ml_dtypes import bfloat16

    xts = [np.ascontiguousarray(x[b].T).astype(bfloat16) for b in range(B)]
    wkvts, wots = [], []
    for g in range(2):
        rows = np.concatenate(
            [
                Wkv[c * D + g * DY + h * HD : c * D + g * DY + (h + 1) * HD]
                for h in range(HPC)
                for c in range(3)
            ],
            axis=0,
        )  # [1536, 1024] rows ordered h-major, c-minor
        wkvts.append(np.ascontiguousarray(rows.T).astype(bfloat16))
        wots.append(
            np.ascontiguousarray(Wo[:, g * DY : (g + 1) * DY].T).astype(np.float16)
        )
    in_maps = []
    for core in range(8):
        b, g = divmod(core, 2)
        in_maps.append({"xt": xts[b], "wkvt": wkvts[g], "wot": wots[g]})
    return in_maps


def gather_out(results, bo):
    out = np.empty((B, N, D), np.float32)
    for b in range(B):
        out[b] = np.asarray(results[2 * b]["out"], np.float32) + np.asarray(
            results[2 * b + 1]["out"], np.float32
        )
    out += bo.astype(np.float32)
    return out


def _numpy_reference(x, mask, Wkv, Wo, bo):
    """Exact fallback (used only if a nonzero additive mask is passed)."""
    x64 = x.astype(np.float64)
    qkv = x64 @ Wkv.T.astype(np.float64)
    q, k, v = np.split(qkv, 3, axis=-1)
    q = q.reshape(B, N, H, HD).transpose(0, 2, 1, 3)
    k = k.reshape(B, N, H, HD).transpose(0, 2, 1, 3)
    v = v.reshape(B, N, H, HD).transpose(0, 2, 1, 3)
    s = q @ k.transpose(0, 1, 3, 2) / np.sqrt(HD) + mask.astype(np.float64)
    s = s - s.max(axis=-1, keepdims=True)
    p = np.exp(s)
    p /= p.sum(axis=-1, keepdims=True)
    y = (p @ v).transpose(0, 2, 1, 3).reshape(B, N, D)
    return (y @ Wo.T.astype(np.float64) + bo.astype(np.float64)).astype(np.float32)


def kernel(x, mask, Wkv, Wo, bo):
    x = np.asarray(x, dtype=np.float32)
    mask = np.asarray(mask, dtype=np.float32)
    Wkv = np.asarray(Wkv, dtype=np.float32)
    Wo = np.asarray(Wo, dtype=np.float32)
    bo = np.asarray(bo, dtype=np.float32)
    if mask.size and np.abs(mask).max() != 0.0:
        return _numpy_reference(x, mask, Wkv, Wo, bo)

    from concourse.bass_utils import run_bass_kernel_spmd

    nc = _get_nc()
    res = run_bass_kernel_spmd(nc, make_in_maps(x, Wkv, Wo), core_ids=list(range(8)))
    return gather_out(res.results, bo)


if __name__ == "__main__":
    rng = np.random.default_rng(0)
    x = rng.standard_normal((B, N, D), dtype=np.float32)
    mask = np.zeros((N, N), np.float32)
    Wkv = (rng.standard_normal((3 * D, D), dtype=np.float32) / np.sqrt(D)).astype(np.float32)
    Wo = (rng.standard_normal((D, D), dtype=np.float32) / np.sqrt(D)).astype(np.float32)
    bo = np.zeros((D,), np.float32)
    got = kernel(x, mask, Wkv, Wo, bo)
    want = _numpy_reference(x, mask, Wkv, Wo, bo)
    err = np.linalg.norm(got - want) / np.linalg.norm(want)
    print("rel err:", err)

